# revision 50
# baseline (speedup 1.0000x reference)
"""DiffUnpool batched GEMM on 8 Trainium2 NeuronCores.

out[b] = S[b] @ x[b] for b in 0..15 (B=16, M=2048, K=256, N=256); A is
passed through unused and never touches the device.

Sharding: pure data parallel over the batch dim - 2 batches per core, no
communication.

Numerics (harness tolerance rel_err < 2e-2, max-abs / max|expected|): the
device computes (S - 0.5) @ (x_hi + x_lo) entirely in fp8 e4m3 with fp32
PSUM accumulate and fp16 stores; the host adds back the rank-1 shift
0.5 * colsum(x_hi + x_lo) and upcasts.  S' = S-0.5 lands in e4m3's
well-covered [-0.5, 0.5] range; x_hi = fp8(x), x_lo = fp8(x - x_hi)
restores x to ~fp16 precision through two accumulating matmuls.  Exact
end-to-end error against the reference inputs: 1.25e-2 (hardware-verified).

Why fp8 everywhere: the kernel is DMA-bound, and the TensorE DoubleRow
perf mode (fp8-only) streams 0.5 cycles/row while contracting both
128-partition k-planes per instruction, so S' moves as 1 byte/elem AND
the PE floor drops to ~3.4 us/core.  Per-core traffic: S' 1 MB + x 0.25
MB + out 2 MB = 3.25 MB (~9.5 us at the modeled 360 GB/s DMA pool, the
binding resource).

DMA-instruction count is the second-order cost (each HWDGE trigger holds
a shared descriptor-gen unit ~630 ns), so all tensors are host-packed
into dense [128, free] layouts with the k-planes interleaved per
512-output-column block, and move in 5 loads + 8 stores per core:
  - st chunks [128, 2*KT, 512] fp8: self-sufficient blocks (both k-planes),
  - xt [128, 4, 512] fp8: (hi|lo, k) stationary planes, one load,
  - loads + all stores ride SP (a store waiting on its copy must never
    head-of-line block the ACT copy stream); x rides ACT at t=0.
Per 512-block: two DoubleRow matmuls (x_hi, x_lo) into a PSUM tile;
middle groups accumulate 1024 cols in a double-bank tile drained by one
wide copy alternating DVE / ACT; the first and last groups split into
per-512 copies/stores on both engine pairs (early DMA-pool store work /
short closing chain).  PE ramp is burned off by ~14 tiny warmup matmuls
on garbage data while the first loads are in flight.
"""

import numpy as np

B, N_ORIG, N_POOL, C = 16, 2048, 256, 256
N_CORES = 8
B_PER_CORE = B // N_CORES
KT = N_POOL // 128      # k-tiles per batch (2)
CT = C // 128           # c-tiles per batch (2)
WCOLS = 1024            # columns per S' load chunk / PSUM tile width
HALVES = N_ORIG // WCOLS  # 2

_cache: dict = {}


def _apply_multiwait_split_patch():
    """This walrus build rejects instructions with >1 sync wait (CoreV3
    setupSyncWait: "Too many sync wait commands"), but Tile's add_semaphores
    stage attaches several.  Post-process the serialized BIR: for each
    instruction with N>1 waits insert N-1 single-wait NoOps right before it
    on the same engine - per-engine program order preserves the semantics."""
    import orjson
    import concourse.bass as bass

    if getattr(bass.Bass, "_mwsplit_patched", False):
        return

    counter = [0]

    def split_multiwait(bir: dict) -> dict:
        for fn in bir.get("functions", []):
            for blk in fn.get("blocks", []):
                out = []
                changed = False
                for inst in blk.get("instructions", []):
                    si = inst.get("sync_info") or {}
                    waits = si.get("on_wait") or []
                    if len(waits) > 1:
                        changed = True
                        for w in waits[:-1]:
                            counter[0] += 1
                            out.append(
                                {
                                    "engine": inst["engine"],
                                    "ins": [],
                                    "outs": [],
                                    "name": f"I-mwsplit-{counter[0]}",
                                    "opcode": "NoOp",
                                    "debug": inst.get("debug", 0),
                                    "sync_info": {"on_update": [], "on_wait": [w]},
                                }
                            )
                        si["on_wait"] = [waits[-1]]
                    out.append(inst)
                if changed:
                    blk["instructions"] = out
        return bir

    orig_bytes = bass.Bass.to_json_bytes

    def to_json_bytes(self) -> bytes:
        return orjson.dumps(split_multiwait(orjson.loads(orig_bytes(self))))

    def to_json_str(self) -> str:
        return to_json_bytes(self).decode()

    def to_json(self) -> dict:
        return orjson.loads(to_json_bytes(self))

    bass.Bass.to_json_bytes = to_json_bytes
    bass.Bass.to_json_str = to_json_str
    bass.Bass.to_json = to_json
    bass.Bass._mwsplit_patched = True


def _build_nc(reps: int = 1):
    import concourse.bass as bass
    import concourse.mybir as mybir
    import concourse.tile as tile

    _apply_multiwait_split_patch()

    f32 = mybir.dt.float32
    f16 = mybir.dt.float16
    f8 = mybir.dt.float8e4
    nc = bass.Bass()
    # Host-packed per-core layouts (p_l = partition, c_l = out partition):
    #   st[p_l, ((b*4+blk)*2 + k)*512 + nl] = (S-0.5)[b, blk*512+nl, k*128+p_l]
    #     (fp8 e4m3; a block = both k-halves for 512 output columns)
    #   xs[p_l, ((which*2 + k)*B + b)*256 + c] = x_{hi,lo}[b, k*128+p_l, c]
    #     (fp8 e4m3 hi/lo split of x)
    #   out[c_l, gi*1024 + m]: group-ordered, gi = (b*HALVES+half)*CT + ct,
    #     n = half*1024 + m                                           (fp16)
    st = nc.declare_dram_parameter(
        "st", [128, KT * B_PER_CORE * N_ORIG], f8, isOutput=False
    )
    xs = nc.declare_dram_parameter(
        "xs", [128, 2 * KT * B_PER_CORE * C], f8, isOutput=False
    )
    out = nc.declare_dram_parameter(
        "out", [128, B_PER_CORE * CT * N_ORIG], f16, isOutput=True
    )

    NBLK = N_ORIG // 512  # 512-col output blocks per batch (4)

    with tile.TileContext(nc) as tc:
        with (
            tc.tile_pool(name="w", bufs=2 * B_PER_CORE * NBLK) as wpool,
            tc.tile_pool(name="xp", bufs=2) as xpool,
            tc.tile_pool(name="ps", bufs=3, space="PSUM") as pspool,
            tc.tile_pool(name="ps5", bufs=2, space="PSUM") as ps5pool,
            tc.tile_pool(name="ob", bufs=6) as opool,
            tc.tile_pool(name="wu", bufs=1) as wupool,
        ):
            # PE warmup: dummy matmuls into a scratch PSUM bank while the
            # first input DMAs are in flight, so the HAM clock-gate ramp
            # (cold 1.2 GHz -> warm 2.4 GHz) burns off before real matmuls.
            # Tiny operands: the memsets ride DVE (idle at t=0, ~0.1 us
            # each) so warmup begins right after the tile-framework prologue
            # and each ~110 ns dummy hands off finely to the first
            # data-dependent matmul.  The warmup accumulator borrows a ps5
            # buffer (PSUM is fully subscribed: 3x2-bank ps + 2x1-bank ps5 =
            # 8 banks); it is released at the warmup stop, well before the
            # first split group needs its second buffer.
            dummy_w = wupool.tile([128, 1], f32, tag="wu_w")
            dummy_x = wupool.tile([128, 32], f32, tag="wu_x")
            nc.vector.memset(dummy_w[:], 1.0)
            nc.vector.memset(dummy_x[:], 1.0)
            wps = ps5pool.tile([128, 512], f32, tag="ps5", name="wps")
            NWU = 14
            for i in range(NWU):
                nc.tensor.matmul(
                    wps[:1, :32],
                    dummy_w[:],
                    dummy_x[:],
                    start=(i == 0),
                    stop=(i == NWU - 1),
                )

            # S' moves in 4 chunks of 2 blocks each (a block = 512 output
            # columns with both k-planes, 1 KB/partition): few HWDGE
            # triggers, and the DoubleRow PE never outruns the arrivals.
            CHUNK_BLKS = [2, 2, 2, 2]
            SPLITS = {0, 7}  # groups with per-512 copies/stores

            for _ in range(reps):
                # x leads the SP queue: every matmul needs it, and SP's
                # HWDGE+dge latency beats ACT's.  xt free dims are
                # (which*2 + k, b*256+c): slicing [:, w2 : w2+2, bc] gives
                # the 3D [p, 2, 128] stationary DoubleRow expects.
                xt = xpool.tile([128, 4, B_PER_CORE * C], f8, tag="x")
                # two half-loads: b0's stationary columns land ~0.7us sooner
                for xb in range(B_PER_CORE):
                    nc.sync.dma_start(
                        out=xt[:, :, xb * C : (xb + 1) * C],
                        in_=xs[:, :].rearrange(
                            "p (w bc) -> p w bc", w=4
                        )[:, :, xb * C : (xb + 1) * C]
                        if False
                        else xs[:, :],
                    )
                wblk = {}  # global blk -> (tile, k-pair index within tile)
                blk0 = 0
                for nblks in CHUNK_BLKS:
                    w = wpool.tile([128, nblks * KT, 512], f8, tag="w", name="w")
                    s0 = blk0 * (KT * 512)
                    nc.sync.dma_start(out=w[:], in_=st[:, s0 : s0 + nblks * KT * 512])
                    for j in range(nblks):
                        wblk[blk0 + j] = (w, j)
                    blk0 += nblks

                # Pool TensorCopy cannot downcast fp32->fp16 (BIR verifier
                # rejects it); DVE and ACT both can.  Queue discipline: SP
                # carries loads then every store (a store waiting on its copy
                # must never head-of-line block the ACT copy stream); each
                # group's two 512-wide halves accumulate in separate PSUM
                # tiles (no WAR between a half's copy and the next half's
                # matmuls) and are copied on DVE (sub 0) / ACT (sub 1) in
                # parallel.  The out DRAM layout is ordered by group index so
                # consecutive groups merge into few big stores; the final
                # group stores its halves separately so the closing chain is
                # one 512-wide hop.
                ngroups = B_PER_CORE * HALVES * CT  # 8

                import os as _os

                USE_DR = _os.environ.get("K_DR", "1") == "1"

                def mm_pair(ct, sub, psap, b, half):
                    # One k-accumulated 512-block: two DoubleRow matmuls
                    # (x_hi then x_lo), each contracting both k-planes at
                    # 0.5 cycles/row.  (K_DR=0: plain fp8 matmuls, same
                    # data layout, for hardware A/B.)
                    blk = (b * HALVES + half) * 2 + sub
                    wt, j = wblk[blk]
                    bc = b * C + ct * 128
                    if USE_DR:
                        for which in range(2):
                            nc.tensor.matmul(
                                psap,
                                xt[:, which * 2 : which * 2 + 2, bc : bc + 128],
                                wt[:, j * KT : (j + 1) * KT, :],
                                start=(which == 0),
                                stop=(which == 1),
                                perf_mode=mybir.MatmulPerfMode.DoubleRow,
                            )
                    else:
                        for which in range(2):
                            for k in range(KT):
                                nc.tensor.matmul(
                                    psap,
                                    xt[:, which * 2 + k : which * 2 + k + 1, bc : bc + 128],
                                    wt[:, j * KT + k : j * KT + k + 1, :],
                                    start=(which == 0 and k == 0),
                                    stop=(which == 1 and k == KT - 1),
                                )

                for b in range(B_PER_CORE):
                    for half in range(HALVES):
                        for ct in range(CT):
                            gi = (b * HALVES + half) * CT + ct
                            # First and last group: per-512 copies/stores on
                            # both engines in parallel (early pool work /
                            # short closing chain).  Middle groups: one
                            # double-bank PSUM tile and a single 1024-wide
                            # copy alternating DVE/ACT - amortizes the PSUM
                            # access latency and halves per-copy overhead,
                            # keeping copy throughput above the DoubleRow
                            # PE's 2x faster group cadence.
                            split = gi in SPLITS
                            if split:
                                for sub in range(WCOLS // 512):
                                    ps = ps5pool.tile(
                                        [128, 512], f32, tag="ps5", name="ps5"
                                    )
                                    mm_pair(ct, sub, ps, b, half)
                                    cp = (nc.vector.tensor_copy, nc.scalar.copy)[
                                        sub % 2
                                    ]
                                    obh = opool.tile(
                                        [128, 512], f16, tag="ob", name="obh"
                                    )
                                    cp(obh[:], ps[:])
                                    # final group: halves exit via SP and ACT
                                    # in parallel (ACT's copy queue is empty
                                    # by then); first group stays on SP while
                                    # ACT is still copying.
                                    sq = nc.sync.dma_start
                                    if gi == ngroups - 1 and sub == 1:
                                        sq = nc.scalar.dma_start
                                    sq(
                                        out=out[:, gi * WCOLS + sub * 512 :][:, :512],
                                        in_=obh[:],
                                    )
                            else:
                                ps = pspool.tile([128, WCOLS], f32, tag="ps", name="ps")
                                for sub in range(WCOLS // 512):
                                    mm_pair(
                                        ct,
                                        sub,
                                        ps[:, sub * 512 : (sub + 1) * 512],
                                        b,
                                        half,
                                    )
                                ob = opool.tile([128, WCOLS], f16, tag="ob", name="ob")
                                cp = (nc.vector.tensor_copy, nc.scalar.copy)[gi % 2]
                                cp(ob[:], ps[:])
                                nc.sync.dma_start(
                                    out=out[:, gi * WCOLS :][:, :WCOLS], in_=ob[:]
                                )
    return nc


def _get_nc():
    if "nc" not in _cache:
        _cache["nc"] = _build_nc()
    return _cache["nc"]


def _pack_x(x: np.ndarray):
    """fp8 e4m3 hi/lo split of x, packed into [N_CORES, 128, 2048]."""
    import ml_dtypes

    e4 = ml_dtypes.float8_e4m3
    x_hi = x.astype(e4)
    x_lo = (x - x_hi.astype(np.float32)).astype(e4)
    xq = np.stack([x_hi, x_lo], axis=0)  # (which, B, P, C)
    xs = (
        xq.reshape(2, N_CORES, B_PER_CORE, KT, 128, C)
        .transpose(1, 4, 0, 3, 2, 5)  # (core, p_l, which, k, b, c)
        .reshape(N_CORES, 128, 2 * KT * B_PER_CORE * C)
    )
    xsum = x_hi.astype(np.float32) + x_lo.astype(np.float32)  # [B, P, C]
    return np.ascontiguousarray(xs), xsum


def _pack_inputs(x: np.ndarray, S: np.ndarray):
    """Host packing into the 2D per-core device layouts."""
    import ml_dtypes

    e4 = ml_dtypes.float8_e4m3
    S8 = (S - np.float32(0.5)).astype(e4)
    # st[core, p_l, ((b*4 + blk)*2 + k)*512 + nl] = S'[core*2+b, blk*512+nl,
    # k*128+p_l]: chunk (b, blk) holds k0|k1 halves for 512 output columns.
    st = (
        S8.reshape(N_CORES, B_PER_CORE, N_ORIG // 512, 512, KT, 128)
        .transpose(0, 5, 1, 2, 4, 3)
        .reshape(N_CORES, 128, KT * B_PER_CORE * N_ORIG)
    )
    xs, xsum = _pack_x(x)
    return np.ascontiguousarray(st), xs, xsum


def _unpack_output(out_dev: np.ndarray, xsum: np.ndarray) -> np.ndarray:
    # out_dev [N_CORES, 128, 8192]; free = gi*1024 + m with
    # gi = (b*HALVES + half)*CT + ct and n = half*1024 + m.
    o = out_dev.reshape(N_CORES, 128, B_PER_CORE, HALVES, CT, WCOLS)
    o = o.transpose(0, 2, 3, 5, 4, 1)  # (core, b, half, m, ct, c_l)
    o = o.reshape(B, N_ORIG, C).astype(np.float32)
    # add back the rank-1 shift: S @ x = (S - 0.5) @ x + 0.5 * colsum(x),
    # using the quantized x the device actually saw.
    corr = 0.5 * xsum.sum(axis=1)  # [B, C]
    return o + corr[:, None, :]


def _run(x: np.ndarray, S: np.ndarray, trace: bool = False):
    from concourse.bass_utils import run_bass_kernel_spmd

    nc = _get_nc()
    st, xs, xsum = _pack_inputs(x, S)
    core_ids = list(range(N_CORES))
    in_maps = [{"st": st[i], "xs": xs[i]} for i in core_ids]
    res = run_bass_kernel_spmd(nc, in_maps, core_ids, trace=trace)
    out_dev = np.stack([res.results[i]["out"] for i in core_ids], axis=0)
    return _unpack_output(out_dev, xsum), res


def kernel(x: np.ndarray, S: np.ndarray, A: np.ndarray = None, **_: dict) -> np.ndarray:
    x = np.asarray(x, dtype=np.float32)
    S = np.asarray(S, dtype=np.float32)
    out, _res = _run(x, S, trace=False)
    return out
